# revision 25
# baseline (speedup 1.0000x reference)
"""Associative-embedding (AE) loss kernel for Trainium2, 8 NeuronCores.

Problem: tags [32, 262144, 1] f32, keypoints [32, 30, 17, 2] int
(col0 = flat heatmap index, col1 = valid flag). Output [32, 2] f32 =
stack([pull, push], axis=1) per batch.

Strategy (pure data parallel, 4 batches per core), v4:
  - Host packs the VALID keypoints of the core's 4 batches densely into
    C = ceil(n_valid/128) slots of 128 partitions; per slot column it
    emits an int32 flat offset and a bf16 assignment matrix
    A[c][slot, person] = valid/cnt.
  - C chained indirect DMAs (standard DGE InstDMACopy -- one offset per
    partition is a hard ucode limit; InstDMAGatherAnt could batch 1024
    descriptors per instruction but costs an ~8us Q7 library load per
    execution, measured slower overall).
  - Per chunk, DVE emits bf16 [v, v^2] and ONE single-pass bf16 matmul
    accumulates [mean_row; m2_row] = sum_c [v,v^2]^T A_c into PSUM
    [2, 120] (the old kernel used two fp32 matmuls per chunk; fp32
    runs the PE array twice, LOW+HIGH).
  - One tiny PE transpose ([2,128] -> [128,2] via a 2x2 identity)
    yields the mean/m2 columns; the pairwise exp argument is built by
    TWO accumulating matmuls: rank-1 mean x mean plus rank-6
    [ones, ones, e_b..] x [-m^2/2, -PEN_OUT/2, r*e_b..], with the
    -m_p^2 term injected via the scalar engine's per-partition
    activation bias. bf16 rounding of the penalty constants is
    compensated exactly on the host (S_EFF / PSCALE2).
  - scalar Exp (scale=2) with free-axis accum gives per-person pull
    sums; DVE removes the diagonal and applies the pull scale; a single
    [128,4]^T @ [128,2] bf16 matmul reduces persons -> [pull, push].

Each core returns its own [4, 2] rows; the host concatenates to [32, 2].
"""

import os
import sys

import numpy as np

if "/opt/trn_rl_repo" not in sys.path:
    sys.path.insert(0, "/opt/trn_rl_repo")

import ml_dtypes

import concourse.bacc as bacc
import concourse.bass as bass
import concourse.tile as tile
from concourse import mybir
from concourse.bass_utils import run_bass_kernel_spmd

# Problem constants (hardcoded per the harness contract)
B, N, D = 32, 262144, 1
P, J = 30, 17
NCORES = 8
BL = B // NCORES          # 4 local batches per core
NFLAT = BL * N            # 1048576 f32 elements in the per-core tags shard
PP = 128                  # slot partitions / person slots (120 real + 8 pad)
NPER = BL * P             # 120 persons per core
PULL_SCALE = 0.5 / (P * (P - 1) / 2.0) * 0.5      # 1/1740
PEN_IN = -float(np.log(PULL_SCALE))               # ~7.46, same-batch offdiag
PEN_OUT = 60.0                                    # exp(-60) == 0 in f32

_F32 = mybir.dt.float32
_I32 = mybir.dt.int32
_BF16 = mybir.dt.bfloat16

# bf16-rounded penalty constants actually seen by the PE, and the exact
# host-side compensation so the final pull scale is unaffected.
_R_BF = float(np.asarray((PEN_OUT - PEN_IN) / 2.0, ml_dtypes.bfloat16))
_C_BF = float(np.asarray(-PEN_OUT / 2.0, ml_dtypes.bfloat16))  # -30, exact
PEN_IN_EFF = -2.0 * (_C_BF + _R_BF)
S_EFF = float(np.exp(-PEN_IN_EFF))      # diagonal exp value to subtract
PSCALE2 = PULL_SCALE / S_EFF            # rescale so same-batch scale is exact


def _build_bass(C: int):
    nc = bacc.Bacc("TRN2", target_bir_lowering=False, debug=False,
                   num_devices=NCORES)

    tags_ext = nc.dram_tensor("tags", [NFLAT, 1], _F32, kind="ExternalInput")
    fidx_ext = nc.dram_tensor("fidx", [PP, C], _I32, kind="ExternalInput")
    a_ext = nc.dram_tensor("amat", [PP, C, NPER], _BF16, kind="ExternalInput")
    hl_ext = nc.dram_tensor("hlmat", [6, PP], _BF16, kind="ExternalInput")
    hr_ext = nc.dram_tensor("hrmat", [6, PP], _BF16, kind="ExternalInput")
    ws_ext = nc.dram_tensor("wsel", [PP, BL], _BF16, kind="ExternalInput")
    out_ext = nc.dram_tensor("out", [BL, 2], _F32, kind="ExternalOutput")

    with tile.TileContext(nc) as tc:
        with tc.tile_pool(name="sb", bufs=1) as pool, \
             tc.tile_pool(name="ps", bufs=1, space="PSUM") as psum:
            # fidx first on sync (it gates the gathers); the big A matrix
            # behind it on the same queue; small constants on scalar.
            fidx_t = pool.tile([PP, C], _I32)
            nc.sync.dma_start(fidx_t[:], fidx_ext[:])
            a_t = pool.tile([PP, C, NPER], _BF16)
            nc.sync.dma_start(a_t[:], a_ext[:])
            hl_t = pool.tile([6, PP], _BF16)
            nc.scalar.dma_start(hl_t[:], hl_ext[:])
            hr_t = pool.tile([6, PP], _BF16)
            nc.scalar.dma_start(hr_t[:], hr_ext[:])
            ws_t = pool.tile([PP, BL], _BF16)
            nc.scalar.dma_start(ws_t[:], ws_ext[:])
            one1 = pool.tile([1, 1], _F32)
            nc.vector.memset(one1[:], 1.0)

            # Warm the scalar engine's Exp table during the gather window.
            zdum = pool.tile([PP, 1], _F32)
            nc.vector.memset(zdum[:], 0.0)
            edum = pool.tile([PP, 1], _F32)
            nc.scalar.activation(edum[:], zdum[:],
                                 mybir.ActivationFunctionType.Exp)

            # Packed gather + accumulate:
            # mm_ps[0:2, person] = sum_c [v_c, v_c^2]^T @ A_c
            v_t = pool.tile([PP, C], _F32)
            rhs2 = pool.tile([PP, C, 2], _BF16)
            mm_ps = psum.tile([2, PP], _F32)
            for c in range(C):
                nc.gpsimd.indirect_dma_start(
                    out=v_t[:, c:c + 1], out_offset=None, in_=tags_ext[:],
                    in_offset=bass.IndirectOffsetOnAxis(
                        ap=fidx_t[:, c:c + 1], axis=0),
                )
                nc.vector.tensor_copy(rhs2[:, c, 0:1], v_t[:, c:c + 1])
                nc.vector.tensor_scalar(
                    out=rhs2[:, c, 1:2], in0=v_t[:, c:c + 1],
                    scalar1=v_t[:, c:c + 1], scalar2=None,
                    op0=mybir.AluOpType.mult,
                )
                nc.tensor.matmul(mm_ps[:, 0:NPER], rhs2[:, c, :],
                                 a_t[:, c, :], start=(c == 0),
                                 stop=(c == C - 1), skip_group_check=True)

            # rows -> SBUF (pad persons zeroed)
            mmrows = pool.tile([2, PP], _BF16)
            nc.vector.memset(mmrows[:], 0.0)
            nc.vector.tensor_copy(mmrows[:, 0:NPER], mm_ps[:, 0:NPER])

            # device row of Hr: zc[q] = -mean[q]^2/2 (partition-0 write)
            nc.vector.scalar_tensor_tensor(
                out=hr_t[0:1, :], in0=mmrows[0:1, :], scalar=-0.5,
                in1=mmrows[0:1, :],
                op0=mybir.AluOpType.mult, op1=mybir.AluOpType.mult,
            )

            # Z[p,q] = mp*mq - mq^2/2 - mp^2/2 - PEN_OUT/2 + r*same(p,q);
            # the -mp^2/2 row rides the zc row as a rank-1 term against the
            # ones row of Hl (no transpose, no per-partition bias needed).
            z_ps = psum.tile([PP, PP], _F32)
            nc.tensor.matmul(z_ps[:], mmrows[0:1, :], mmrows[0:1, :],
                             start=True, stop=False, skip_group_check=True)
            nc.tensor.matmul(z_ps[:], hr_t[0:1, :], hl_t[0:1, :],
                             start=False, stop=False, skip_group_check=True)
            nc.tensor.matmul(z_ps[:], hl_t[:], hr_t[:], start=False,
                             stop=True, skip_group_check=True)

            # push rows (runs parallel to the PE/scalar path):
            # push_b[b] = sum_{p in b} (m2[p] - mean[p]^2) / P
            # m2 sits on partition 1 (engine reads must start at partition
            # 0), so hop it to a partition-0 tile with an SBUF->SBUF DMA.
            m2row = pool.tile([1, PP], _BF16)
            nc.sync.dma_start(m2row[:], mmrows[1:2, :])
            sqrow = pool.tile([1, PP], _F32)
            nc.vector.scalar_tensor_tensor(
                out=sqrow[:], in0=mmrows[0:1, :], scalar=-1.0 / P,
                in1=mmrows[0:1, :],
                op0=mybir.AluOpType.mult, op1=mybir.AluOpType.mult,
            )
            pushr = pool.tile([1, BL, P], _F32)
            nc.vector.scalar_tensor_tensor(
                out=pushr[:, :, :], in0=m2row[0:1, 0:NPER], scalar=1.0 / P,
                in1=sqrow[0:1, 0:NPER],
                op0=mybir.AluOpType.mult, op1=mybir.AluOpType.add,
            )
            push_b = pool.tile([1, BL], _F32)
            nc.vector.tensor_reduce(push_b[:], pushr[:, :, :],
                                    axis=mybir.AxisListType.X,
                                    op=mybir.AluOpType.add)
            out_ps = psum.tile([BL, 2], _F32)
            nc.tensor.matmul(out_ps[:, 1:2], push_b[:], one1[:], start=True,
                             stop=True, skip_group_check=True)

            # exp(2Z) with free-axis accumulation -> per-person pull sums
            e_t = pool.tile([PP, PP], _BF16)
            x0 = pool.tile([PP, 1], _F32)
            nc.scalar.activation(e_t[:], z_ps[:],
                                 mybir.ActivationFunctionType.Exp, scale=2.0,
                                 accum_out=x0[:])

            # pull column: drop the diagonal exp(-PEN_IN_EFF), apply scale
            x0f = pool.tile([PP, 1], _BF16)
            nc.vector.tensor_scalar(
                out=x0f[:], in0=x0[:], scalar1=S_EFF, scalar2=PSCALE2,
                op0=mybir.AluOpType.subtract, op1=mybir.AluOpType.mult,
            )
            nc.tensor.matmul(out_ps[:, 0:1], ws_t[:], x0f[:], start=True,
                             stop=True, skip_group_check=True)

            res = pool.tile([BL, 2], _F32)
            nc.vector.tensor_copy(res[:], out_ps[:])
            nc.sync.dma_start(out_ext[:], res[:])

    nc.compile()
    return nc


def _prep_core_inputs(core: int, tags: np.ndarray, kp: np.ndarray,
                      C: int) -> dict:
    """Host-side preprocessing: shard + packed index/assignment tables."""
    b0 = core * BL
    t = np.ascontiguousarray(
        tags[b0:b0 + BL].reshape(NFLAT, 1).astype(np.float32, copy=False))

    idx = kp[b0:b0 + BL, :, :, 0].astype(np.int64)       # [BL,P,J]
    val = (kp[b0:b0 + BL, :, :, 1] == 1)                 # [BL,P,J]
    cnt = np.maximum(val.sum(-1), 1).astype(np.float32)  # [BL,P]

    flat = (idx + np.arange(BL)[:, None, None] * N)      # [BL,P,J] < NFLAT
    person = np.broadcast_to(
        np.arange(NPER).reshape(BL, P, 1), (BL, P, J))

    vflat = flat[val]          # [n_valid] gather offsets
    vperson = person[val]      # [n_valid] owning person
    wval = (1.0 / cnt.reshape(NPER))[vperson]            # weight 1/cnt
    n_valid = vflat.shape[0]
    assert n_valid <= C * PP

    fidx = np.zeros((PP, C), np.int32)
    amat = np.zeros((PP, C, NPER), np.float32)
    s = np.arange(n_valid)
    prt, call = s % PP, s // PP
    fidx[prt, call] = vflat
    amat[prt, call, vperson] = wval

    ebs = np.zeros((BL, PP), np.float32)
    for b in range(BL):
        ebs[b, b * P:(b + 1) * P] = 1.0
    hl = np.zeros((6, PP), np.float32)
    hl[0, :] = 1.0                 # pairs the device zc row
    hl[1, :] = 1.0                 # pairs the constant row
    hl[2:6] = ebs
    hr = np.zeros((6, PP), np.float32)
    hr[1, :] = -PEN_OUT / 2.0
    hr[2:6] = _R_BF * ebs

    return {"tags": t, "fidx": fidx,
            "amat": amat.astype(ml_dtypes.bfloat16),
            "hlmat": hl.astype(ml_dtypes.bfloat16),
            "hrmat": hr.astype(ml_dtypes.bfloat16),
            "wsel": np.ascontiguousarray(ebs.T).astype(ml_dtypes.bfloat16)}


_NC_CACHE = {}


def _get_nc(C: int):
    if C not in _NC_CACHE:
        _NC_CACHE[C] = _build_bass(C)
    return _NC_CACHE[C]


def _ensure_profile_hook():
    """Provide antenv.axon_hooks if the image's antenv lacks it, so
    run_bass_kernel_spmd(trace=True) can capture NTFF profiles under axon.
    Mirrors trn_agent_boot's ctypes shim over libaxon_pjrt.so."""
    try:
        from antenv.axon_hooks import get_axon_ntff_profile_hook  # noqa: F401
        return
    except ImportError:
        pass
    import contextlib
    import ctypes
    import types

    so_path = "/opt/axon/libaxon_pjrt.so"
    if not os.path.exists(so_path):
        return
    lib = ctypes.CDLL(so_path)
    if not hasattr(lib, "axon_start_nrt_profile"):
        return
    lib.axon_start_nrt_profile.argtypes = [ctypes.POINTER(ctypes.c_int64),
                                           ctypes.c_size_t]
    lib.axon_start_nrt_profile.restype = ctypes.c_int64
    lib.axon_stop_nrt_profile.argtypes = [ctypes.c_char_p]
    lib.axon_stop_nrt_profile.restype = ctypes.c_int64

    @contextlib.contextmanager
    def _hook(output_dir, device_ids):
        import jax
        jax.devices()
        if device_ids:
            ids = (ctypes.c_int64 * len(device_ids))(*device_ids)
            rc = lib.axon_start_nrt_profile(ids, len(device_ids))
        else:
            rc = lib.axon_start_nrt_profile(None, 0)
        if rc != 0:
            raise RuntimeError(f"axon_start_nrt_profile rc={rc}")
        try:
            yield
        finally:
            n = lib.axon_stop_nrt_profile(str(output_dir).encode())
            print(f"profile: {n} file(s) written to {output_dir}",
                  file=sys.stderr)

    mod = types.ModuleType("antenv.axon_hooks")
    _state = {"hook": _hook}
    mod.set_axon_ntff_profile_hook = lambda h: _state.__setitem__("hook", h)
    mod.get_axon_ntff_profile_hook = lambda: _state["hook"]
    sys.modules["antenv.axon_hooks"] = mod


def run(tags: np.ndarray, keypoints: np.ndarray, **spmd_kwargs):
    """Build in_maps, run on 8 cores, return ([32,2] f32, BassKernelResults)."""
    tags = np.asarray(tags)
    kp = np.asarray(keypoints)
    if spmd_kwargs.get("trace"):
        _ensure_profile_hook()
    val = (kp[..., 1] == 1).reshape(NCORES, -1)
    C = max(1, int(np.ceil(val.sum(axis=1).max() / PP)))
    nc = _get_nc(C)
    in_maps = [_prep_core_inputs(c, tags, kp, C) for c in range(NCORES)]
    results = run_bass_kernel_spmd(nc, in_maps, core_ids=list(range(NCORES)),
                                   **spmd_kwargs)
    out = np.concatenate([np.asarray(results.results[c]["out"])
                          for c in range(NCORES)], axis=0)
    return out.astype(np.float32), results


def kernel(tags: np.ndarray, keypoints: np.ndarray) -> np.ndarray:
    out, _ = run(tags, keypoints)
    return out


# revision 31
# speedup vs baseline: 1.0337x; 1.0337x over previous
"""Associative-embedding (AE) loss kernel for Trainium2, 8 NeuronCores.

Problem: tags [32, 262144, 1] f32, keypoints [32, 30, 17, 2] int
(col0 = flat heatmap index, col1 = valid flag). Output [32, 2] f32 =
stack([pull, push], axis=1) per batch.

Strategy (pure data parallel, 4 batches per core), v4:
  - Host packs the VALID keypoints of the core's 4 batches densely into
    C = ceil(n_valid/128) slots of 128 partitions; per slot column it
    emits an int32 flat offset and a bf16 assignment matrix
    A[c][slot, person] = valid/cnt.
  - C chained indirect DMAs (standard DGE InstDMACopy -- one offset per
    partition is a hard ucode limit; InstDMAGatherAnt could batch 1024
    descriptors per instruction but costs an ~8us Q7 library load per
    execution, measured slower overall).
  - Per chunk, DVE emits bf16 [v, v^2] and ONE single-pass bf16 matmul
    accumulates [mean_row; m2_row] = sum_c [v,v^2]^T A_c into PSUM
    [2, 120] (the old kernel used two fp32 matmuls per chunk; fp32
    runs the PE array twice, LOW+HIGH).
  - One tiny PE transpose ([2,128] -> [128,2] via a 2x2 identity)
    yields the mean/m2 columns; the pairwise exp argument is built by
    TWO accumulating matmuls: rank-1 mean x mean plus rank-6
    [ones, ones, e_b..] x [-m^2/2, -PEN_OUT/2, r*e_b..], with the
    -m_p^2 term injected via the scalar engine's per-partition
    activation bias. bf16 rounding of the penalty constants is
    compensated exactly on the host (S_EFF / PSCALE2).
  - scalar Exp (scale=2) with free-axis accum gives per-person pull
    sums; DVE removes the diagonal and applies the pull scale; a single
    [128,4]^T @ [128,2] bf16 matmul reduces persons -> [pull, push].

Each core returns its own [4, 2] rows; the host concatenates to [32, 2].
"""

import os
import sys

import numpy as np

if "/opt/trn_rl_repo" not in sys.path:
    sys.path.insert(0, "/opt/trn_rl_repo")

import ml_dtypes

import concourse.bacc as bacc
import concourse.bass as bass
import concourse.tile as tile
from concourse import mybir
from concourse.bass_utils import run_bass_kernel_spmd

# Problem constants (hardcoded per the harness contract)
B, N, D = 32, 262144, 1
P, J = 30, 17
NCORES = 8
BL = B // NCORES          # 4 local batches per core
NFLAT = BL * N            # 1048576 f32 elements in the per-core tags shard
PP = 128                  # slot partitions / person slots (120 real + 8 pad)
NPER = BL * P             # 120 persons per core
PULL_SCALE = 0.5 / (P * (P - 1) / 2.0) * 0.5      # 1/1740
PEN_IN = -float(np.log(PULL_SCALE))               # ~7.46, same-batch offdiag
PEN_OUT = 60.0                                    # exp(-60) == 0 in f32

_F32 = mybir.dt.float32
_I32 = mybir.dt.int32
_BF16 = mybir.dt.bfloat16

# bf16-rounded penalty constants actually seen by the PE, and the exact
# host-side compensation so the final pull scale is unaffected.
_R_BF = float(np.asarray((PEN_OUT - PEN_IN) / 2.0, ml_dtypes.bfloat16))
_C_BF = float(np.asarray(-PEN_OUT / 2.0, ml_dtypes.bfloat16))  # -30, exact
PEN_IN_EFF = -2.0 * (_C_BF + _R_BF)
S_EFF = float(np.exp(-PEN_IN_EFF))      # diagonal exp value to subtract
PSCALE2 = PULL_SCALE / S_EFF            # rescale so same-batch scale is exact


def _build_bass(C: int):
    nc = bacc.Bacc("TRN2", target_bir_lowering=False, debug=False,
                   num_devices=NCORES)

    tags_ext = nc.dram_tensor("tags", [NFLAT, 1], _F32, kind="ExternalInput")
    fidx_ext = nc.dram_tensor("fidx", [PP, C], _I32, kind="ExternalInput")
    a_ext = nc.dram_tensor("amat", [PP, C, NPER], _BF16, kind="ExternalInput")
    ab_ext = nc.dram_tensor("abat", [PP, C, BL], _BF16, kind="ExternalInput")
    hl_ext = nc.dram_tensor("hlmat", [6, PP], _BF16, kind="ExternalInput")
    hr_ext = nc.dram_tensor("hrmat", [6, PP], _BF16, kind="ExternalInput")
    ws_ext = nc.dram_tensor("wsel", [PP, BL], _BF16, kind="ExternalInput")
    out_ext = nc.dram_tensor("out", [BL, 2], _F32, kind="ExternalOutput")

    with tile.TileContext(nc) as tc:
        with tc.tile_pool(name="sb", bufs=1) as pool, \
             tc.tile_pool(name="ps", bufs=1, space="PSUM") as psum:
            # fidx first on sync (it gates the gathers); the big A matrix
            # behind it on the same queue; small constants on scalar.
            fidx_t = pool.tile([PP, C], _I32)
            nc.sync.dma_start(fidx_t[:], fidx_ext[:])
            a_t = pool.tile([PP, C, NPER], _BF16)
            nc.sync.dma_start(a_t[:], a_ext[:])
            ab_t = pool.tile([PP, C, BL], _BF16)
            nc.scalar.dma_start(ab_t[:], ab_ext[:])
            hl_t = pool.tile([6, PP], _BF16)
            nc.scalar.dma_start(hl_t[:], hl_ext[:])
            hr_t = pool.tile([6, PP], _BF16)
            nc.scalar.dma_start(hr_t[:], hr_ext[:])
            ws_t = pool.tile([PP, BL], _BF16)
            nc.scalar.dma_start(ws_t[:], ws_ext[:])
            one1 = pool.tile([1, 1], _F32)
            nc.vector.memset(one1[:], 1.0)

            # Warm the scalar engine's Exp table during the gather window.
            zdum = pool.tile([PP, 1], _F32)
            nc.vector.memset(zdum[:], 0.0)
            edum = pool.tile([PP, 1], _F32)
            nc.scalar.activation(edum[:], zdum[:],
                                 mybir.ActivationFunctionType.Exp)

            # Packed gather + accumulate:
            # mm_ps[0:2, person] = sum_c [v_c, v_c^2]^T @ A_c
            v_t = pool.tile([PP, C], _F32)
            rhs2 = pool.tile([PP, C, 2], _BF16)
            mm_ps = psum.tile([2, PP], _F32)
            m2b_ps = psum.tile([1, BL], _F32)
            for c in range(C):
                nc.gpsimd.indirect_dma_start(
                    out=v_t[:, c:c + 1], out_offset=None, in_=tags_ext[:],
                    in_offset=bass.IndirectOffsetOnAxis(
                        ap=fidx_t[:, c:c + 1], axis=0),
                )
                nc.vector.tensor_copy(rhs2[:, c, 0:1], v_t[:, c:c + 1])
                nc.vector.tensor_scalar(
                    out=rhs2[:, c, 1:2], in0=v_t[:, c:c + 1],
                    scalar1=v_t[:, c:c + 1], scalar2=None,
                    op0=mybir.AluOpType.mult,
                )
                nc.tensor.matmul(mm_ps[:, 0:NPER], rhs2[:, c, :],
                                 a_t[:, c, :], start=(c == 0),
                                 stop=(c == C - 1), skip_group_check=True)
                # per-batch m2 partial sums on partition 0 (readable later;
                # mm_ps row 1 is partition 1, which engines cannot address)
                nc.tensor.matmul(m2b_ps[:, :], rhs2[:, c, 1:2],
                                 ab_t[:, c, :], start=(c == 0),
                                 stop=(c == C - 1), skip_group_check=True)

            # rows -> SBUF (pad persons zeroed)
            mmrows = pool.tile([2, PP], _BF16)
            nc.vector.memset(mmrows[:], 0.0)
            nc.vector.tensor_copy(mmrows[:, 0:NPER], mm_ps[:, 0:NPER])

            # device row of Hr: zc[q] = -mean[q]^2/2 (partition-0 write)
            nc.vector.scalar_tensor_tensor(
                out=hr_t[0:1, :], in0=mmrows[0:1, :], scalar=-0.5,
                in1=mmrows[0:1, :],
                op0=mybir.AluOpType.mult, op1=mybir.AluOpType.mult,
            )

            # Z[p,q] = mp*mq - mq^2/2 - mp^2/2 - PEN_OUT/2 + r*same(p,q);
            # the -mp^2/2 row rides the zc row as a rank-1 term against the
            # ones row of Hl (no transpose, no per-partition bias needed).
            z_ps = psum.tile([PP, PP], _F32)
            nc.tensor.matmul(z_ps[:], mmrows[0:1, :], mmrows[0:1, :],
                             start=True, stop=False, skip_group_check=True)
            nc.tensor.matmul(z_ps[:], hr_t[0:1, :], hl_t[0:1, :],
                             start=False, stop=False, skip_group_check=True)
            nc.tensor.matmul(z_ps[:], hl_t[:], hr_t[:], start=False,
                             stop=True, skip_group_check=True)

            # push (runs parallel to the PE/scalar path):
            # push_b[b] = (m2b[b] - sum_{p in b} mean[p]^2) / P
            sqrow = pool.tile([1, BL, P], _F32)
            nc.vector.scalar_tensor_tensor(
                out=sqrow[:, :, :], in0=mmrows[0:1, 0:NPER],
                scalar=-1.0 / P, in1=mmrows[0:1, 0:NPER],
                op0=mybir.AluOpType.mult, op1=mybir.AluOpType.mult,
            )
            msqb = pool.tile([1, BL], _F32)
            nc.vector.tensor_reduce(msqb[:], sqrow[:, :, :],
                                    axis=mybir.AxisListType.X,
                                    op=mybir.AluOpType.add)
            push_b = pool.tile([1, BL], _F32)
            nc.vector.scalar_tensor_tensor(
                out=push_b[:], in0=m2b_ps[0:1, :], scalar=1.0 / P,
                in1=msqb[:],
                op0=mybir.AluOpType.mult, op1=mybir.AluOpType.add,
            )
            out_ps = psum.tile([BL, 2], _F32)
            nc.tensor.matmul(out_ps[:, 1:2], push_b[:], one1[:], start=True,
                             stop=True, skip_group_check=True)

            # exp(2Z) with free-axis accumulation -> per-person pull sums
            e_t = pool.tile([PP, PP], _BF16)
            x0 = pool.tile([PP, 1], _F32)
            nc.scalar.activation(e_t[:], z_ps[:],
                                 mybir.ActivationFunctionType.Exp, scale=2.0,
                                 accum_out=x0[:])

            # pull column: drop the diagonal exp(-PEN_IN_EFF), apply scale
            x0f = pool.tile([PP, 1], _BF16)
            nc.vector.tensor_scalar(
                out=x0f[:], in0=x0[:], scalar1=S_EFF, scalar2=PSCALE2,
                op0=mybir.AluOpType.subtract, op1=mybir.AluOpType.mult,
            )
            nc.tensor.matmul(out_ps[:, 0:1], ws_t[:], x0f[:], start=True,
                             stop=True, skip_group_check=True)

            res = pool.tile([BL, 2], _F32)
            nc.vector.tensor_copy(res[:], out_ps[:])
            nc.sync.dma_start(out_ext[:], res[:])

    nc.compile()
    return nc


def _prep_core_inputs(core: int, tags: np.ndarray, kp: np.ndarray,
                      C: int) -> dict:
    """Host-side preprocessing: shard + packed index/assignment tables."""
    b0 = core * BL
    t = np.ascontiguousarray(
        tags[b0:b0 + BL].reshape(NFLAT, 1).astype(np.float32, copy=False))

    idx = kp[b0:b0 + BL, :, :, 0].astype(np.int64)       # [BL,P,J]
    val = (kp[b0:b0 + BL, :, :, 1] == 1)                 # [BL,P,J]
    cnt = np.maximum(val.sum(-1), 1).astype(np.float32)  # [BL,P]

    flat = (idx + np.arange(BL)[:, None, None] * N)      # [BL,P,J] < NFLAT
    person = np.broadcast_to(
        np.arange(NPER).reshape(BL, P, 1), (BL, P, J))

    vflat = flat[val]          # [n_valid] gather offsets
    vperson = person[val]      # [n_valid] owning person
    wval = (1.0 / cnt.reshape(NPER))[vperson]            # weight 1/cnt
    n_valid = vflat.shape[0]
    assert n_valid <= C * PP

    fidx = np.zeros((PP, C), np.int32)
    amat = np.zeros((PP, C, NPER), np.float32)
    abat = np.zeros((PP, C, BL), np.float32)
    s = np.arange(n_valid)
    prt, call = s % PP, s // PP
    fidx[prt, call] = vflat
    amat[prt, call, vperson] = wval
    abat[prt, call, vperson // P] = wval

    ebs = np.zeros((BL, PP), np.float32)
    for b in range(BL):
        ebs[b, b * P:(b + 1) * P] = 1.0
    hl = np.zeros((6, PP), np.float32)
    hl[0, :] = 1.0                 # pairs the device zc row
    hl[1, :] = 1.0                 # pairs the constant row
    hl[2:6] = ebs
    hr = np.zeros((6, PP), np.float32)
    hr[1, :] = -PEN_OUT / 2.0
    hr[2:6] = _R_BF * ebs

    return {"tags": t, "fidx": fidx,
            "amat": amat.astype(ml_dtypes.bfloat16),
            "abat": abat.astype(ml_dtypes.bfloat16),
            "hlmat": hl.astype(ml_dtypes.bfloat16),
            "hrmat": hr.astype(ml_dtypes.bfloat16),
            "wsel": np.ascontiguousarray(ebs.T).astype(ml_dtypes.bfloat16)}


_NC_CACHE = {}


def _get_nc(C: int):
    if C not in _NC_CACHE:
        _NC_CACHE[C] = _build_bass(C)
    return _NC_CACHE[C]


def _ensure_profile_hook():
    """Provide antenv.axon_hooks if the image's antenv lacks it, so
    run_bass_kernel_spmd(trace=True) can capture NTFF profiles under axon.
    Mirrors trn_agent_boot's ctypes shim over libaxon_pjrt.so."""
    try:
        from antenv.axon_hooks import get_axon_ntff_profile_hook  # noqa: F401
        return
    except ImportError:
        pass
    import contextlib
    import ctypes
    import types

    so_path = "/opt/axon/libaxon_pjrt.so"
    if not os.path.exists(so_path):
        return
    lib = ctypes.CDLL(so_path)
    if not hasattr(lib, "axon_start_nrt_profile"):
        return
    lib.axon_start_nrt_profile.argtypes = [ctypes.POINTER(ctypes.c_int64),
                                           ctypes.c_size_t]
    lib.axon_start_nrt_profile.restype = ctypes.c_int64
    lib.axon_stop_nrt_profile.argtypes = [ctypes.c_char_p]
    lib.axon_stop_nrt_profile.restype = ctypes.c_int64

    @contextlib.contextmanager
    def _hook(output_dir, device_ids):
        import jax
        jax.devices()
        if device_ids:
            ids = (ctypes.c_int64 * len(device_ids))(*device_ids)
            rc = lib.axon_start_nrt_profile(ids, len(device_ids))
        else:
            rc = lib.axon_start_nrt_profile(None, 0)
        if rc != 0:
            raise RuntimeError(f"axon_start_nrt_profile rc={rc}")
        try:
            yield
        finally:
            n = lib.axon_stop_nrt_profile(str(output_dir).encode())
            print(f"profile: {n} file(s) written to {output_dir}",
                  file=sys.stderr)

    mod = types.ModuleType("antenv.axon_hooks")
    _state = {"hook": _hook}
    mod.set_axon_ntff_profile_hook = lambda h: _state.__setitem__("hook", h)
    mod.get_axon_ntff_profile_hook = lambda: _state["hook"]
    sys.modules["antenv.axon_hooks"] = mod


def run(tags: np.ndarray, keypoints: np.ndarray, **spmd_kwargs):
    """Build in_maps, run on 8 cores, return ([32,2] f32, BassKernelResults)."""
    tags = np.asarray(tags)
    kp = np.asarray(keypoints)
    if spmd_kwargs.get("trace"):
        _ensure_profile_hook()
    val = (kp[..., 1] == 1).reshape(NCORES, -1)
    C = max(1, int(np.ceil(val.sum(axis=1).max() / PP)))
    nc = _get_nc(C)
    in_maps = [_prep_core_inputs(c, tags, kp, C) for c in range(NCORES)]
    results = run_bass_kernel_spmd(nc, in_maps, core_ids=list(range(NCORES)),
                                   **spmd_kwargs)
    out = np.concatenate([np.asarray(results.results[c]["out"])
                          for c in range(NCORES)], axis=0)
    return out.astype(np.float32), results


def kernel(tags: np.ndarray, keypoints: np.ndarray) -> np.ndarray:
    out, _ = run(tags, keypoints)
    return out
